# revision 1
# baseline (speedup 1.0000x reference)
"""LinearAttention Trainium2 kernel: data-parallel over batch on 8 NeuronCores.

Reference computation per batch b (C=256 channels, L=4096 seq, H=8 heads, D=64):
  qkv = w_qkv @ x[b]                    # (1536, L)
  q, k, v = split(qkv)                  # each (512, L), rows = (head, dim)
  k = softmax(k, axis=L)
  ctx[h] = k[h] @ v[h].T                # (64, 64)
  out[h] = ctx[h].T @ q[h]              # (64, L)
  y[b] = w_out @ concat(out) + b_out    # (256, L)

Per-core design (2 batches/core):
  - K^T, V^T computed with L on partitions (lhsT = x chunk, rhs = w^T) so the
    context matmul contracts over L on the TensorEngine.
  - context computed TRANSPOSED per head-pair: ctxT[e,d] = sum_l v[e,l]exp(k[d,l])
    (lhsT = v^T chunk, rhs = expk^T chunk), cross-head quadrants discarded via
    a zeroed block-diagonal SBUF tile.
  - w_out is folded into the context on the PE: McT[d,o] = sum_e ctxT[e,d]wo[e,o],
    which removes the separate attention-out matmul and its PSUM->SBUF copies.
    The softmax denominator (row matmul with a ones lhsT, then 4 tiny PE
    transposes) is applied as a per-partition ACT scale on the McT copy.
  - final: y = McT.T @ q + b, contracting the 512 q-channels in 4 chunks.
  - exp() applied unshifted (inputs are N(0,1)-scaled; max |k| ~ 5, safe in f32).
  - all TensorE compute in bf16 (f32 PSUM accumulation).
"""

import numpy as np

B, C, L = 16, 256, 4096
HID = 512
N_CORES = 8
NB = B // N_CORES  # batches per core
CC = C // 128  # contraction chunks for the input projections (2)
LP = L // 128  # l-tiles with l on partitions (32)
LT = L // 512  # l-tiles of 512 for moving-dim matmuls (8)
PR = HID // 128  # head-pairs (4): each 128-wide chunk = 2 heads of 64

_CACHE = {}


def _build(reps=1):
    from concourse import bacc, mybir, tile
    import concourse.bass as bass

    bf16 = mybir.dt.bfloat16
    f32 = mybir.dt.float32
    Exp = mybir.ActivationFunctionType.Exp
    Copy = mybir.ActivationFunctionType.Copy
    Ident = mybir.ActivationFunctionType.Identity

    nc = bacc.Bacc(
        "TRN2",
        target_bir_lowering=False,
        debug=False,
        enable_asserts=False,
        num_devices=N_CORES,
    )

    x_d = nc.dram_tensor("x", [NB, CC, 128, L], bf16, kind="ExternalInput")
    wq_d = nc.dram_tensor("wq_t", [CC, 128, HID], bf16, kind="ExternalInput")
    wk_d = nc.dram_tensor("wk_t", [CC, 128, HID], bf16, kind="ExternalInput")
    wv_d = nc.dram_tensor("wv_t", [CC, 128, HID], bf16, kind="ExternalInput")
    wo_d = nc.dram_tensor("wo_t", [PR, 128, C], bf16, kind="ExternalInput")
    bb_d = nc.dram_tensor("bb", [128, 2], f32, kind="ExternalInput")
    out_d = nc.dram_tensor("out", [NB, 2, 128, L], f32, kind="ExternalOutput")

    with tile.TileContext(nc) as tc:
        with (
            tc.tile_pool(name="const", bufs=1) as const,
            tc.tile_pool(name="xp", bufs=2) as xp,
            tc.tile_pool(name="big", bufs=1) as big,
            tc.tile_pool(name="small", bufs=2) as small,
            tc.tile_pool(name="qtp", bufs=4) as qtp,
            tc.tile_pool(name="ostp", bufs=3) as ostp,
            tc.tile_pool(name="ps_mm", bufs=3, space="PSUM") as ps_mm,
            tc.tile_pool(name="ps_ctx", bufs=4, space="PSUM") as ps_ctx,
            tc.tile_pool(name="ps_den", bufs=1, space="PSUM") as ps_den,
        ):
            wq = const.tile([128, CC, HID], bf16)
            wk = const.tile([128, CC, HID], bf16)
            wv = const.tile([128, CC, HID], bf16)
            wo = const.tile([128, PR, C], bf16)
            bb = const.tile([128, 2], f32)
            ones_col = const.tile([128, 1], bf16)
            id11 = const.tile([1, 1], f32)
            ctxt_sb = const.tile([128, PR, 128], bf16)

            for cc in range(CC):
                nc.sync.dma_start(wq[:, cc, :], wq_d[cc])
                nc.sync.dma_start(wk[:, cc, :], wk_d[cc])
                nc.sync.dma_start(wv[:, cc, :], wv_d[cc])
            for pr in range(PR):
                nc.sync.dma_start(wo[:, pr, :], wo_d[pr])
            nc.sync.dma_start(bb[:], bb_d[:])
            nc.gpsimd.memset(ones_col[:], 1.0)
            nc.gpsimd.memset(id11[:], 1.0)
            nc.gpsimd.memset(ctxt_sb[:], 0.0)

            for rep in range(reps):
              for bi in range(NB):
                xt = xp.tile([128, CC, L], bf16)
                for cc in range(CC):
                    nc.sync.dma_start(xt[:, cc, :], x_d[bi, cc])

                expkt = big.tile([128, LP, HID], bf16, tag="expkt")
                vt = big.tile([128, LP, HID], bf16, tag="vt")

                # K^T / V^T projections fused with the transposed-context and
                # denominator accumulations: PE streams without phase breaks.
                ctx_p = [
                    ps_ctx.tile([128, 128], f32, tag="ctx", name=f"ctx_{rep}_{bi}_{g}")
                    for g in range(PR)
                ]
                den_ps = ps_den.tile([1, HID], f32, tag="den")
                for lp in range(LP):
                    psk = ps_mm.tile([128, HID], f32, tag="mm")
                    psv = ps_mm.tile([128, HID], f32, tag="mm")
                    for cc in range(CC):
                        nc.tensor.matmul(
                            psk[:],
                            xt[:, cc, lp * 128 : (lp + 1) * 128],
                            wk[:, cc, :],
                            start=(cc == 0),
                            stop=(cc == CC - 1),
                        )
                    for cc in range(CC):
                        nc.tensor.matmul(
                            psv[:],
                            xt[:, cc, lp * 128 : (lp + 1) * 128],
                            wv[:, cc, :],
                            start=(cc == 0),
                            stop=(cc == CC - 1),
                        )
                    nc.scalar.activation(expkt[:, lp, :], psk[:], Exp)
                    nc.vector.tensor_copy(vt[:, lp, :], psv[:])
                    for pr in range(PR):
                        nc.tensor.matmul(
                            ctx_p[pr][:],
                            vt[:, lp, pr * 128 : (pr + 1) * 128],
                            expkt[:, lp, pr * 128 : (pr + 1) * 128],
                            start=(lp == 0),
                            stop=(lp == LP - 1),
                        )
                    nc.tensor.matmul(
                        den_ps[:],
                        ones_col[:],
                        expkt[:, lp, :],
                        start=(lp == 0),
                        stop=(lp == LP - 1),
                        skip_group_check=True,
                    )
                den_sb = small.tile([1, HID], f32, tag="densb")
                nc.vector.tensor_copy(den_sb[:], den_ps[:])
                tps = ps_mm.tile([128, PR], f32, tag="mm")
                for pr in range(PR):
                    nc.tensor.transpose(
                        tps[:, pr : pr + 1],
                        den_sb[0:1, pr * 128 : (pr + 1) * 128],
                        id11[:],
                    )
                inv_den = small.tile([128, PR], f32, tag="invden")
                nc.vector.reciprocal(inv_den[:], tps[:])

                # block-diagonal ctxT (cross-head quadrants stay zero).
                for pr in range(PR):
                    nc.vector.tensor_copy(
                        ctxt_sb[0:64, pr, 0:64], ctx_p[pr][0:64, 0:64]
                    )
                    nc.vector.tensor_copy(
                        ctxt_sb[64:128, pr, 64:128], ctx_p[pr][64:128, 64:128]
                    )
                # fold w_out into the context: McT[d, o], scaled by 1/den[d].
                mct = small.tile([128, PR, C], bf16, tag="mct")
                for pr in range(PR):
                    mc_ps = ps_mm.tile([128, C], f32, tag="mm")
                    nc.tensor.matmul(
                        mc_ps[:], ctxt_sb[:, pr, :], wo[:, pr, :], start=True, stop=True
                    )
                    nc.scalar.activation(
                        mct[:, pr, :], mc_ps[:], Copy, scale=inv_den[:, pr : pr + 1]
                    )

                # Q projection + fused output projection, per l-chunk of 512.
                for lt in range(LT):
                    qt = qtp.tile([128, PR, 512], bf16, tag="qt")
                    for oc in range(PR):
                        psq = ps_mm.tile([128, 512], f32, tag="mm")
                        for cc in range(CC):
                            nc.tensor.matmul(
                                psq[:],
                                wq[:, cc, oc * 128 : (oc + 1) * 128],
                                xt[:, cc, lt * 512 : (lt + 1) * 512],
                                start=(cc == 0),
                                stop=(cc == CC - 1),
                            )
                        nc.vector.tensor_copy(qt[:, oc, :], psq[:])
                    ostg = ostp.tile([128, 2, 512], f32, tag="ostg")
                    for oc2 in range(2):
                        psf = ps_mm.tile([128, 512], f32, tag="mm")
                        for pr in range(PR):
                            nc.tensor.matmul(
                                psf[:],
                                mct[:, pr, oc2 * 128 : (oc2 + 1) * 128],
                                qt[:, pr, :],
                                start=(pr == 0),
                                stop=(pr == PR - 1),
                            )
                        nc.scalar.activation(
                            ostg[:, oc2, :],
                            psf[:],
                            Ident,
                            bias=bb[:, oc2 : oc2 + 1],
                        )
                        nc.sync.dma_start(
                            out_d[bi, oc2, :, lt * 512 : (lt + 1) * 512],
                            ostg[:, oc2, :],
                        )

    nc.compile()
    return nc


def _get_nc():
    if "nc" not in _CACHE:
        _CACHE["nc"] = _build()
    return _CACHE["nc"]


def _prep_in_maps(x, w_qkv, w_out, b_out):
    import ml_dtypes

    bf16 = ml_dtypes.bfloat16
    wq_t = np.ascontiguousarray(w_qkv[0:512].T).reshape(CC, 128, HID).astype(bf16)
    wk_t = np.ascontiguousarray(w_qkv[512:1024].T).reshape(CC, 128, HID).astype(bf16)
    wv_t = np.ascontiguousarray(w_qkv[1024:1536].T).reshape(CC, 128, HID).astype(bf16)
    wo_t = np.ascontiguousarray(w_out.T).reshape(PR, 128, C).astype(bf16)
    bb = np.ascontiguousarray(b_out.reshape(2, 128).T).astype(np.float32)
    in_maps = []
    for c in range(N_CORES):
        xs = x[c * NB : (c + 1) * NB].reshape(NB, CC, 128, L).astype(bf16)
        in_maps.append(
            {
                "x": np.ascontiguousarray(xs),
                "wq_t": wq_t,
                "wk_t": wk_t,
                "wv_t": wv_t,
                "wo_t": wo_t,
                "bb": bb,
            }
        )
    return in_maps


def kernel(x, w_qkv, w_out, b_out):
    from concourse.bass_utils import run_bass_kernel_spmd

    nc = _get_nc()
    in_maps = _prep_in_maps(
        np.asarray(x, dtype=np.float32),
        np.asarray(w_qkv, dtype=np.float32),
        np.asarray(w_out, dtype=np.float32),
        np.asarray(b_out, dtype=np.float32),
    )
    res = run_bass_kernel_spmd(nc, in_maps, core_ids=list(range(N_CORES)))
    out = np.concatenate(
        [res.results[c]["out"].reshape(NB, C, L) for c in range(N_CORES)], axis=0
    )
    return out.astype(np.float32)



# revision 4
# speedup vs baseline: 22.2213x; 22.2213x over previous
"""LinearAttention Trainium2 kernel: data-parallel over batch on 8 NeuronCores.

Reference computation per batch b (C=256 channels, L=4096 seq, H=8 heads, D=64):
  qkv = w_qkv @ x[b]                    # (1536, L)
  q, k, v = split(qkv)                  # each (512, L), rows = (head, dim)
  k = softmax(k, axis=L)
  ctx[h] = k[h] @ v[h].T                # (64, 64)
  out[h] = ctx[h].T @ q[h]              # (64, L)
  y[b] = w_out @ concat(out) + b_out    # (256, L)

Key algebraic optimization: the attention output is LINEAR in q, so
  y[b] = w_out @ ctx^T @ (Wq @ x[b]) + b = (w_out @ ctx^T @ Wq) @ x[b] + b
       = MW[b] @ x[b] + b,   MW[b] a per-batch (256, 256) matrix.
This removes the Q projection GEMM and shrinks the output GEMM contraction
from 512 to 256 (PE columns per batch drop from ~161K to ~100K).

Per-core design (2 batches/core):
  - K^T, V^T computed with L on partitions (lhsT = x chunk, rhs = w^T) so the
    context matmul contracts over L on the TensorEngine.
  - context computed TRANSPOSED per head-pair: ctxT[e,d] = sum_l v[e,l]exp(k[d,l])
    (lhsT = v^T chunk, rhs = expk^T chunk), cross-head quadrants discarded via
    a zeroed block-diagonal SBUF tile. ctx/den matmuls lag the K/V projections
    by one l-tile so the PE never waits on the ACT engine's exp.
  - softmax denominator via tiny N=1 matmuls (lhsT = expk^T chunk, rhs = ones)
    accumulating across l-tiles -> den lands with d on partitions directly
    (no 512-wide row-sum matmul, no transposes).
  - w_out folded into the context on the PE: McT[d,o] = sum_e ctxT[e,d]wo[e,o],
    scaled by 1/den[d] on the ACT copy; then MW^T[ci,co] = sum_d wq[d,ci]McT[d,co].
  - final: y = MW^T.T @ x + b, contracting the 256 input channels in 2 chunks;
    bias applied via a 1-row accumulating matmul (lhsT = bias row, rhs = ones).
  - exp() applied unshifted (inputs are N(0,1)-scaled; max |k| ~ 5, safe in f32).
  - all TensorE compute in bf16 (f32 PSUM accumulation).
"""

import numpy as np

B, C, L = 16, 256, 4096
HID = 512
N_CORES = 8
NB = B // N_CORES  # batches per core
CC = C // 128  # contraction chunks for the input projections (2)
LP = L // 128  # l-tiles with l on partitions (32)
LT = L // 512  # l-tiles of 512 for moving-dim matmuls (8)
PR = HID // 128  # head-pairs (4): each 128-wide chunk = 2 heads of 64

_CACHE = {}


def _build(reps=1):
    from concourse import bacc, mybir, tile
    import concourse.bass as bass

    bf16 = mybir.dt.bfloat16
    f32 = mybir.dt.float32
    Exp = mybir.ActivationFunctionType.Exp
    Copy = mybir.ActivationFunctionType.Copy

    nc = bacc.Bacc(
        "TRN2",
        target_bir_lowering=False,
        debug=False,
        enable_asserts=False,
        num_devices=N_CORES,
    )

    x_d = nc.dram_tensor("x", [NB, CC, 128, L], bf16, kind="ExternalInput")
    wk_d = nc.dram_tensor("wk_t", [CC, 128, HID], bf16, kind="ExternalInput")
    wv_d = nc.dram_tensor("wv_t", [CC, 128, HID], bf16, kind="ExternalInput")
    wqd_d = nc.dram_tensor("wqd", [PR, 128, C], bf16, kind="ExternalInput")
    wo_d = nc.dram_tensor("wo_t", [PR, 128, C], bf16, kind="ExternalInput")
    bias_d = nc.dram_tensor("bias", [1, C], bf16, kind="ExternalInput")
    out_d = nc.dram_tensor("out", [NB, 2, 128, L], f32, kind="ExternalOutput")

    with tile.TileContext(nc) as tc:
        with (
            tc.tile_pool(name="const", bufs=1) as const,
            tc.tile_pool(name="xp", bufs=2) as xp,
            tc.tile_pool(name="big", bufs=1) as big,
            tc.tile_pool(name="small", bufs=2) as small,
            tc.tile_pool(name="ostp", bufs=3) as ostp,
            tc.tile_pool(name="ps_kv", bufs=3, space="PSUM") as ps_kv,
            tc.tile_pool(name="ps_ctx", bufs=1, space="PSUM") as ps_ctx,
            tc.tile_pool(name="ps_den", bufs=1, space="PSUM") as ps_den,
            tc.tile_pool(name="ps_out", bufs=2, space="PSUM") as ps_out,
        ):
            wk = const.tile([128, CC, HID], bf16)
            wv = const.tile([128, CC, HID], bf16)
            wqd = const.tile([128, PR, C], bf16)
            wo = const.tile([128, PR, C], bf16)
            bias_sb = const.tile([1, C], bf16)
            ones_col = const.tile([128, 1], bf16)
            ones_row = const.tile([1, 512], bf16)
            ctxt_sb = const.tile([128, PR, 128], bf16)

            for cc in range(CC):
                nc.sync.dma_start(wk[:, cc, :], wk_d[cc])
                nc.sync.dma_start(wv[:, cc, :], wv_d[cc])
            for pr in range(PR):
                nc.sync.dma_start(wqd[:, pr, :], wqd_d[pr])
                nc.sync.dma_start(wo[:, pr, :], wo_d[pr])
            nc.sync.dma_start(bias_sb[:], bias_d[:])
            nc.gpsimd.memset(ones_col[:], 1.0)
            nc.gpsimd.memset(ones_row[:], 1.0)
            nc.gpsimd.memset(ctxt_sb[:], 0.0)

            for rep in range(reps):
              for bi in range(NB):
                xt = xp.tile([128, CC, L], bf16)
                for cc in range(CC):
                    nc.sync.dma_start(xt[:, cc, :], x_d[bi, cc])

                expkt = big.tile([128, LP, HID], bf16, tag="expkt")
                vt = big.tile([128, LP, HID], bf16, tag="vt")
                ctx_ps = ps_ctx.tile(
                    [128, PR, 128], f32, tag="ctx", name=f"ctx_{rep}_{bi}"
                )
                den_ps = ps_den.tile(
                    [128, PR], f32, tag="den", name=f"den_{rep}_{bi}"
                )

                def ctx_den(lp):
                    # start=True marks the tile's whole 2KB PSUM bank
                    # "pending-zero"; each matmul's first write then
                    # overwrites its own bytes. So only the chronologically
                    # first matmul per bank may carry start=True — siblings
                    # would re-mark the bank and wipe earlier partial sums.
                    # transposed per-pair context, contracting this l-tile
                    for pr in range(PR):
                        nc.tensor.matmul(
                            ctx_ps[:, pr, :],
                            vt[:, lp, pr * 128 : (pr + 1) * 128],
                            expkt[:, lp, pr * 128 : (pr + 1) * 128],
                            start=(lp == 0 and pr == 0),
                            stop=(lp == LP - 1),
                            skip_group_check=True,
                        )
                    # denominator: den[d] += sum_l expk[d,l], d on partitions
                    for pr in range(PR):
                        nc.tensor.matmul(
                            den_ps[:, pr : pr + 1],
                            expkt[:, lp, pr * 128 : (pr + 1) * 128],
                            ones_col[:],
                            start=(lp == 0 and pr == 0),
                            stop=(lp == LP - 1),
                            skip_group_check=True,
                        )

                # K^T / V^T projections; ctx/den lag one l-tile so the exp()
                # for tile lp runs while the PE projects tile lp+1.
                for lp in range(LP):
                    psk = ps_kv.tile([128, HID], f32, tag="kv")
                    psv = ps_kv.tile([128, HID], f32, tag="kv")
                    for cc in range(CC):
                        nc.tensor.matmul(
                            psk[:],
                            xt[:, cc, lp * 128 : (lp + 1) * 128],
                            wk[:, cc, :],
                            start=(cc == 0),
                            stop=(cc == CC - 1),
                        )
                    for cc in range(CC):
                        nc.tensor.matmul(
                            psv[:],
                            xt[:, cc, lp * 128 : (lp + 1) * 128],
                            wv[:, cc, :],
                            start=(cc == 0),
                            stop=(cc == CC - 1),
                        )
                    nc.scalar.activation(expkt[:, lp, :], psk[:], Exp)
                    nc.vector.tensor_copy(vt[:, lp, :], psv[:])
                    if lp > 0:
                        ctx_den(lp - 1)
                ctx_den(LP - 1)

                inv_den = small.tile([128, PR], f32, tag="invden")
                nc.vector.reciprocal(inv_den[:], den_ps[:])

                # block-diagonal ctxT (cross-head quadrants stay zero).
                for pr in range(PR):
                    nc.vector.tensor_copy(
                        ctxt_sb[0:64, pr, 0:64], ctx_ps[0:64, pr, 0:64]
                    )
                    nc.vector.tensor_copy(
                        ctxt_sb[64:128, pr, 64:128], ctx_ps[64:128, pr, 64:128]
                    )
                # fold w_out into the context: McT[d, o], scaled by 1/den[d].
                mct = small.tile([128, PR, C], bf16, tag="mct")
                for pr in range(PR):
                    mc_ps = ps_out.tile([128, C], f32, tag="out")
                    nc.tensor.matmul(
                        mc_ps[:], ctxt_sb[:, pr, :], wo[:, pr, :], start=True, stop=True
                    )
                    nc.scalar.activation(
                        mct[:, pr, :], mc_ps[:], Copy, scale=inv_den[:, pr : pr + 1]
                    )
                # fold Wq: MW^T[ci, co] = sum_d wq[d, ci] * McT[d, co]
                mwt_ps = ps_out.tile([128, 2, C], f32, tag="out")
                for c2 in range(2):
                    for pr in range(PR):
                        nc.tensor.matmul(
                            mwt_ps[:, c2, :],
                            wqd[:, pr, c2 * 128 : (c2 + 1) * 128],
                            mct[:, pr, :],
                            # c2=1's first write consumes pending-zero bytes
                            # left by c2=0's start (same bank) — see ctx_den.
                            start=(c2 == 0 and pr == 0),
                            stop=(pr == PR - 1),
                            skip_group_check=True,
                        )
                mwt = small.tile([128, 2, C], bf16, tag="mwt")
                nc.vector.tensor_copy(mwt[:], mwt_ps[:])

                # final: y = MW^T.T @ x + bias, per l-chunk of 512.
                for lt in range(LT):
                    for oc2 in range(2):
                        psf = ps_out.tile([128, 512], f32, tag="out")
                        for c2 in range(2):
                            nc.tensor.matmul(
                                psf[:],
                                mwt[:, c2, oc2 * 128 : (oc2 + 1) * 128],
                                xt[:, c2, lt * 512 : (lt + 1) * 512],
                                start=(c2 == 0),
                                stop=False,
                            )
                        nc.tensor.matmul(
                            psf[:],
                            bias_sb[0:1, oc2 * 128 : (oc2 + 1) * 128],
                            ones_row[0:1, :],
                            start=False,
                            stop=True,
                        )
                        ostg = ostp.tile([128, 512], f32, tag="ostg")
                        nc.vector.tensor_copy(ostg[:], psf[:])
                        nc.sync.dma_start(
                            out_d[bi, oc2, :, lt * 512 : (lt + 1) * 512],
                            ostg[:],
                        )

    nc.compile()
    return nc


def _get_nc():
    if "nc" not in _CACHE:
        _CACHE["nc"] = _build()
    return _CACHE["nc"]


def _prep_in_maps(x, w_qkv, w_out, b_out):
    import ml_dtypes

    bf16 = ml_dtypes.bfloat16
    wk_t = np.ascontiguousarray(w_qkv[512:1024].T).reshape(CC, 128, HID).astype(bf16)
    wv_t = np.ascontiguousarray(w_qkv[1024:1536].T).reshape(CC, 128, HID).astype(bf16)
    wqd = np.ascontiguousarray(w_qkv[0:512]).reshape(PR, 128, C).astype(bf16)
    wo_t = np.ascontiguousarray(w_out.T).reshape(PR, 128, C).astype(bf16)
    bias = np.ascontiguousarray(b_out.reshape(1, C)).astype(bf16)
    in_maps = []
    for c in range(N_CORES):
        xs = x[c * NB : (c + 1) * NB].reshape(NB, CC, 128, L).astype(bf16)
        in_maps.append(
            {
                "x": np.ascontiguousarray(xs),
                "wk_t": wk_t,
                "wv_t": wv_t,
                "wqd": wqd,
                "wo_t": wo_t,
                "bias": bias,
            }
        )
    return in_maps


def kernel(x, w_qkv, w_out, b_out):
    from concourse.bass_utils import run_bass_kernel_spmd

    nc = _get_nc()
    in_maps = _prep_in_maps(
        np.asarray(x, dtype=np.float32),
        np.asarray(w_qkv, dtype=np.float32),
        np.asarray(w_out, dtype=np.float32),
        np.asarray(b_out, dtype=np.float32),
    )
    res = run_bass_kernel_spmd(nc, in_maps, core_ids=list(range(N_CORES)))
    out = np.concatenate(
        [res.results[c]["out"].reshape(NB, C, L) for c in range(N_CORES)], axis=0
    )
    return out.astype(np.float32)


# revision 10
# speedup vs baseline: 62.1215x; 2.7956x over previous
"""LinearAttention Trainium2 kernel: data-parallel over batch on 8 NeuronCores.

Reference computation per batch b (C=256 channels, L=4096 seq, H=8 heads, D=64):
  qkv = w_qkv @ x[b]                    # (1536, L)
  q, k, v = split(qkv)                  # each (512, L), rows = (head, dim)
  k = softmax(k, axis=L)
  ctx[h] = k[h] @ v[h].T                # (64, 64)
  out[h] = ctx[h].T @ q[h]              # (64, L)
  y[b] = w_out @ concat(out) + b_out    # (256, L)

Key algebraic optimization: the attention output is LINEAR in q, so
  y[b] = w_out @ ctx^T @ (Wq @ x[b]) + b = (w_out @ ctx^T @ Wq) @ x[b] + b
       = MW[b] @ x[b] + b,   MW[b] a per-batch (256, 256) matrix.
This removes the Q projection GEMM and shrinks the output GEMM contraction
from 512 to 256 (PE columns per batch drop from ~161K to ~100K).

Per-core design (2 batches/core):
  - K^T, V^T computed with L on partitions (lhsT = x chunk, rhs = w^T) so the
    context matmul contracts over L on the TensorEngine.
  - context computed TRANSPOSED per head-pair: ctxT[e,d] = sum_l v[e,l]exp(k[d,l])
    (lhsT = v^T chunk, rhs = expk^T chunk), cross-head quadrants discarded via
    a zeroed block-diagonal SBUF tile. ctx/den matmuls lag the K/V projections
    by one l-tile so the PE never waits on the ACT engine's exp.
  - softmax denominator via tiny N=1 matmuls (lhsT = expk^T chunk, rhs = ones)
    accumulating across l-tiles -> den lands with d on partitions directly
    (no 512-wide row-sum matmul, no transposes).
  - w_out folded into the context on the PE: McT[d,o] = sum_e ctxT[e,d]wo[e,o],
    scaled by 1/den[d] on the ACT copy; then MW^T[ci,co] = sum_d wq[d,ci]McT[d,co].
  - final: y = MW^T.T @ x + b, contracting the 256 input channels in 2 chunks;
    bias applied via a 1-row accumulating matmul (lhsT = bias row, rhs = ones).
  - exp() applied unshifted (inputs are N(0,1)-scaled; max |k| ~ 5, safe in f32).
  - all TensorE compute in bf16 (f32 PSUM accumulation).
"""

import numpy as np

B, C, L = 16, 256, 4096
HID = 512
N_CORES = 8
NB = B // N_CORES  # batches per core
CC = C // 128  # contraction chunks for the input projections (2)
LP = L // 128  # l-tiles with l on partitions (32)
LT = L // 512  # l-tiles of 512 for moving-dim matmuls (8)
PR = HID // 128  # head-pairs (4): each 128-wide chunk = 2 heads of 64

_CACHE = {}


def _build(reps=1, with_bias=True):
    from concourse import bacc, mybir, tile
    import concourse.bass as bass

    bf16 = mybir.dt.bfloat16
    f32 = mybir.dt.float32
    Exp = mybir.ActivationFunctionType.Exp
    Copy = mybir.ActivationFunctionType.Copy

    nc = bacc.Bacc(
        "TRN2",
        target_bir_lowering=False,
        debug=False,
        enable_asserts=False,
        num_devices=N_CORES,
    )

    x_d = nc.dram_tensor("x", [NB, CC, 128, L], bf16, kind="ExternalInput")
    wk_d = nc.dram_tensor("wk_t", [CC, 128, HID], bf16, kind="ExternalInput")
    wv_d = nc.dram_tensor("wv_t", [CC, 128, HID], bf16, kind="ExternalInput")
    wqd_d = nc.dram_tensor("wqd", [PR, 128, C], bf16, kind="ExternalInput")
    wo_d = nc.dram_tensor("wo_t", [PR, 128, C], bf16, kind="ExternalInput")
    bias_d = nc.dram_tensor("bias", [1, C], bf16, kind="ExternalInput")
    out_d = nc.dram_tensor("out", [NB, 2, 128, L], f32, kind="ExternalOutput")

    with tile.TileContext(nc) as tc:
        with (
            tc.tile_pool(name="const", bufs=1) as const,
            tc.tile_pool(name="xp", bufs=2) as xp,
            tc.tile_pool(name="big", bufs=1) as big,
            tc.tile_pool(name="small", bufs=2) as small,
            tc.tile_pool(name="ostp", bufs=3) as ostp,
            tc.tile_pool(name="ps_kv", bufs=3, space="PSUM") as ps_kv,
            tc.tile_pool(name="ps_ctx", bufs=1, space="PSUM") as ps_ctx,
            tc.tile_pool(name="ps_den", bufs=1, space="PSUM") as ps_den,
            tc.tile_pool(name="ps_out", bufs=2, space="PSUM") as ps_out,
        ):
            wk = const.tile([128, CC, HID], bf16)
            wv = const.tile([128, CC, HID], bf16)
            wqd = const.tile([128, PR, C], bf16)
            wo = const.tile([128, PR, C], bf16)
            bias_sb = const.tile([1, C], bf16)
            ones_col = const.tile([128, 1], bf16)
            ones_row = const.tile([1, 512], bf16)
            ctxt_sb = const.tile([128, PR, 128], bf16)

            for cc in range(CC):
                nc.sync.dma_start(wk[:, cc, :], wk_d[cc])
                nc.sync.dma_start(wv[:, cc, :], wv_d[cc])
            for pr in range(PR):
                nc.sync.dma_start(wqd[:, pr, :], wqd_d[pr])
                nc.sync.dma_start(wo[:, pr, :], wo_d[pr])
            nc.sync.dma_start(bias_sb[:], bias_d[:])
            nc.gpsimd.memset(ones_col[:], 1.0)
            nc.gpsimd.memset(ones_row[:], 1.0)
            nc.gpsimd.memset(ctxt_sb[:], 0.0)

            for rep in range(reps):
              for bi in range(NB):
                xt = xp.tile([128, CC, L], bf16)
                for cc in range(CC):
                    nc.sync.dma_start(xt[:, cc, :], x_d[bi, cc])

                expkt = big.tile([128, LP, HID], bf16, tag="expkt")
                vt = big.tile([128, LP, HID], bf16, tag="vt")
                ctx_ps = ps_ctx.tile(
                    [128, PR, 128], f32, tag="ctx", name=f"ctx_{rep}_{bi}"
                )
                den_ps = ps_den.tile(
                    [128, PR], f32, tag="den", name=f"den_{rep}_{bi}"
                )

                def ctx_den(lp):
                    # start=True marks the tile's whole 2KB PSUM bank
                    # "pending-zero"; each matmul's first write then
                    # overwrites its own bytes. So only the chronologically
                    # first matmul per bank may carry start=True — siblings
                    # would re-mark the bank and wipe earlier partial sums.
                    # transposed per-pair context, contracting this l-tile
                    for pr in range(PR):
                        nc.tensor.matmul(
                            ctx_ps[:, pr, :],
                            vt[:, lp, pr * 128 : (pr + 1) * 128],
                            expkt[:, lp, pr * 128 : (pr + 1) * 128],
                            start=(lp == 0 and pr == 0),
                            stop=(lp == LP - 1),
                            skip_group_check=True,
                        )
                    # denominator: den[d] += sum_l expk[d,l], d on partitions
                    for pr in range(PR):
                        nc.tensor.matmul(
                            den_ps[:, pr : pr + 1],
                            expkt[:, lp, pr * 128 : (pr + 1) * 128],
                            ones_col[:],
                            start=(lp == 0 and pr == 0),
                            stop=(lp == LP - 1),
                            skip_group_check=True,
                        )

                # K^T / V^T projections; ctx/den lag one l-tile so the exp()
                # for tile lp runs while the PE projects tile lp+1.
                for lp in range(LP):
                    psk = ps_kv.tile([128, HID], f32, tag="kv")
                    psv = ps_kv.tile([128, HID], f32, tag="kv")
                    # K and V share each x-chunk stationary back-to-back so
                    # the PE can skip/overlap the second weight load.
                    for cc in range(CC):
                        nc.tensor.matmul(
                            psk[:],
                            xt[:, cc, lp * 128 : (lp + 1) * 128],
                            wk[:, cc, :],
                            start=(cc == 0),
                            stop=(cc == CC - 1),
                            skip_group_check=True,
                        )
                        nc.tensor.matmul(
                            psv[:],
                            xt[:, cc, lp * 128 : (lp + 1) * 128],
                            wv[:, cc, :],
                            start=(cc == 0),
                            stop=(cc == CC - 1),
                            skip_group_check=True,
                        )
                    nc.scalar.activation(expkt[:, lp, :], psk[:], Exp)
                    nc.vector.tensor_copy(vt[:, lp, :], psv[:])
                    # lag ctx by TWO l-tiles: exp(lp) costs ~720ns on ACT but
                    # the PE covers only ~640ns between psk(lp) stop and a
                    # 1-lagged ctx(lp) — a 2-tile lag gives ~1.7us of slack.
                    if lp > 1:
                        ctx_den(lp - 2)
                ctx_den(LP - 2)
                ctx_den(LP - 1)

                inv_den = small.tile([128, PR], f32, tag="invden")
                nc.vector.reciprocal(inv_den[:], den_ps[:])

                # block-diagonal ctxT (cross-head quadrants stay zero).
                for pr in range(PR):
                    nc.vector.tensor_copy(
                        ctxt_sb[0:64, pr, 0:64], ctx_ps[0:64, pr, 0:64]
                    )
                    nc.vector.tensor_copy(
                        ctxt_sb[64:128, pr, 64:128], ctx_ps[64:128, pr, 64:128]
                    )
                # fold w_out into the context: McT[d, o], scaled by 1/den[d].
                mct = small.tile([128, PR, C], bf16, tag="mct")
                for pr in range(PR):
                    mc_ps = ps_out.tile([128, C], f32, tag="out")
                    nc.tensor.matmul(
                        mc_ps[:], ctxt_sb[:, pr, :], wo[:, pr, :], start=True, stop=True
                    )
                    nc.scalar.activation(
                        mct[:, pr, :], mc_ps[:], Copy, scale=inv_den[:, pr : pr + 1]
                    )
                # fold Wq: MW^T[ci, co] = sum_d wq[d, ci] * McT[d, co]
                mwt_ps = ps_out.tile([128, 2, C], f32, tag="out")
                for c2 in range(2):
                    for pr in range(PR):
                        nc.tensor.matmul(
                            mwt_ps[:, c2, :],
                            wqd[:, pr, c2 * 128 : (c2 + 1) * 128],
                            mct[:, pr, :],
                            # c2=1's first write consumes pending-zero bytes
                            # left by c2=0's start (same bank) — see ctx_den.
                            start=(c2 == 0 and pr == 0),
                            stop=(pr == PR - 1),
                            skip_group_check=True,
                        )
                mwt = small.tile([128, 2, C], bf16, tag="mwt")
                nc.vector.tensor_copy(mwt[:], mwt_ps[:])

                # final: y = MW^T.T @ x + bias, per l-chunk of 512.
                for lt in range(LT):
                    for oc2 in range(2):
                        psf = ps_out.tile([128, 512], f32, tag="out")
                        for c2 in range(2):
                            nc.tensor.matmul(
                                psf[:],
                                mwt[:, c2, oc2 * 128 : (oc2 + 1) * 128],
                                xt[:, c2, lt * 512 : (lt + 1) * 512],
                                start=(c2 == 0),
                                stop=(c2 == 1 and not with_bias),
                            )
                        if with_bias:
                            nc.tensor.matmul(
                                psf[:],
                                bias_sb[0:1, oc2 * 128 : (oc2 + 1) * 128],
                                ones_row[0:1, :],
                                start=False,
                                stop=True,
                            )
                        ostg = ostp.tile([128, 512], f32, tag="ostg")
                        nc.vector.tensor_copy(ostg[:], psf[:])
                        nc.sync.dma_start(
                            out_d[bi, oc2, :, lt * 512 : (lt + 1) * 512],
                            ostg[:],
                        )

    nc.compile()
    return nc


def _get_nc(with_bias=True):
    key = ("nc", with_bias)
    if key not in _CACHE:
        _CACHE[key] = _build(with_bias=with_bias)
    return _CACHE[key]


def _prep_in_maps(x, w_qkv, w_out, b_out):
    import ml_dtypes

    bf16 = ml_dtypes.bfloat16
    wk_t = np.ascontiguousarray(w_qkv[512:1024].T).reshape(CC, 128, HID).astype(bf16)
    wv_t = np.ascontiguousarray(w_qkv[1024:1536].T).reshape(CC, 128, HID).astype(bf16)
    wqd = np.ascontiguousarray(w_qkv[0:512]).reshape(PR, 128, C).astype(bf16)
    wo_t = np.ascontiguousarray(w_out.T).reshape(PR, 128, C).astype(bf16)
    bias = np.ascontiguousarray(b_out.reshape(1, C)).astype(bf16)
    in_maps = []
    for c in range(N_CORES):
        xs = x[c * NB : (c + 1) * NB].reshape(NB, CC, 128, L).astype(bf16)
        in_maps.append(
            {
                "x": np.ascontiguousarray(xs),
                "wk_t": wk_t,
                "wv_t": wv_t,
                "wqd": wqd,
                "wo_t": wo_t,
                "bias": bias,
            }
        )
    return in_maps


def kernel(x, w_qkv, w_out, b_out):
    from concourse.bass_utils import run_bass_kernel_spmd

    # the bias matmuls cost ~12us/rep of PE time; skip them when b_out == 0
    with_bias = bool(np.any(np.asarray(b_out)))
    nc = _get_nc(with_bias=with_bias)
    in_maps = _prep_in_maps(
        np.asarray(x, dtype=np.float32),
        np.asarray(w_qkv, dtype=np.float32),
        np.asarray(w_out, dtype=np.float32),
        np.asarray(b_out, dtype=np.float32),
    )
    res = run_bass_kernel_spmd(nc, in_maps, core_ids=list(range(N_CORES)))
    out = np.concatenate(
        [res.results[c]["out"].reshape(NB, C, L) for c in range(N_CORES)], axis=0
    )
    return out.astype(np.float32)
